# revision 40
# baseline (speedup 1.0000x reference)
"""Trainium2 Bass kernel for nn_CCR_59193239273568 (3-pass spatial attention block).

Strategy (8 NeuronCores, SPMD), v2:
  - Each core owns an 8-image-row band (512 px) of BOTH samples.
  - Phase A: per-band double-conv q/k/v projections (im2col 9-tap fp32r
    matmuls, software-pipelined across tensors so the PE never stalls on the
    DVE relu chain), band outputs written as fp8e4m3 [C, 3, 512] and
    AllGathered per sample (K-forms only, ~50KB/rank/sample -- the collective
    cost model is out-bytes-dominated with a 15us floor, so byte-shrinking
    and per-sample splitting are the big levers).
  - The transposed V^T chunk form [128 keys, 4, 32ch] is rebuilt locally
    post-gather with PE transposes (fp8 needs an output element step of 2,
    hence the [.., 2] psum batch tile) + one DVE copy per (tensor, rank)
    that converts to bf16.  s1's rebuild is emitted mid-B(s0) so it's off
    the critical path.
  - Phase B: per (sample, pass) unit the core computes its 512 query rows of
    softmax(scale * Q^T K) V^T in S^T layout: S^T chunks [128 keys, 512
    queries] as K=32 fp8 matmuls grouped GS=2 per PSUM tile (3-deep slot
    rotation with distinct pool tags so Tile doesn't serialize on false
    WAR deps), exp on ScalarE straight out of PSUM emitting bf16 for 12 of
    16 groups; the last 4 groups are computed on DVE with a Schraudolph
    bf16 bit-trick (bits = round(x*128/ln2 + 16249.1) as int16, reinterpreted
    as bf16; the ~2% sawtooth washes out in the softmax average).  ctx^T +
    rowsum accumulate with [128, 33] stationary (V^T | ones) bf16 operands.
    Normalization via DVE reciprocal + GpSimd partition_broadcast writes the
    normalized band bf16 directly into the phase-C conv input tile (cpad) --
    no DRAM round trip.
  - Halo exchange for the phase-C convs is a per-sample AllToAll of just the
    3-row band edges (each core addresses its top/bottom edge rows to the
    neighbor that needs them; edge cores skip the wraparound write so
    neighbors read SAME-padding zeros).
  - Phase C: wr/wg/wb convs on cpad (bf16 weights), average (1/3 folded into
    w2's ctx input channels host-side), concat with x, w2/w3 in fp32r, write
    the fp32 output band.  SAME-padding edge effects are fixed with per-core
    host "bias images" (-1e30 on out-of-image rows, so the conv relu zeroes
    them).
"""

import sys

import numpy as np

sys.path.insert(0, "/opt/trn_rl_repo")

import concourse.bacc as bacc
import concourse.bass as bass
import concourse.mybir as mybir
import concourse.tile as tile
from concourse.bass_utils import run_bass_kernel_spmd

F32 = mybir.dt.float32
F32R = mybir.dt.float32r
BF16 = mybir.dt.bfloat16
FP8 = mybir.dt.float8e4
AF = mybir.ActivationFunctionType
ALU = mybir.AluOpType

B, CIN, C, H, W = 2, 64, 32, 64, 64
R = 8                 # cores
BR = H // R           # 8 band rows per core per sample
PX = BR * W           # 512 band pixels
N = H * W             # 4096
SCALE = float(C) ** -0.5
NCH = N // 128        # 32 key chunks per sample
GS = 2                # exp group size in chunks (PSUM: 3x2 banks + 2 ctx banks)

KF = 3 * C * PX       # 49152: per-rank K-form elems (t-major [t, c, px])
TSZ = C * PX          # 16384 per tensor

# per-conv metadata: (weight pack name, cin, bias column, bf16?)
CONVS = {
    "q1": ("wq1", 64, 0, False), "q2": ("wq2", 32, 1, False),
    "k1": ("wk1", 64, 2, False), "k2": ("wk2", 32, 3, False),
    "v1": ("wv1", 64, 4, False), "v2": ("wv2", 32, 5, False),
    "r": ("wr", 32, 6, True), "g": ("wg", 32, 7, True), "b": ("wb", 32, 8, True),
    "2": ("w2", 96, 9, False), "3": ("w3", 32, 10, False),
}


def build_program():
    nc = bacc.Bacc("TRN2", target_bir_lowering=False, debug=False, num_devices=R)

    xband_d = nc.declare_dram_parameter("xband", [CIN, B, 12, 66], F32, isOutput=False)
    wd = {}
    for key, (wname, cin, _bi, is_bf) in CONVS.items():
        wd[key] = nc.declare_dram_parameter(
            "p_" + wname, [cin, 9, C], BF16 if is_bf else F32, isOutput=False
        )
    bias_d = nc.declare_dram_parameter("biases", [C, 11], F32, isOutput=False)
    biasA_d = nc.declare_dram_parameter("biasA", [C, 3, 10, W], F32, isOutput=False)
    biasC_d = nc.declare_dram_parameter("biasC", [C, 3, 12, W], F32, isOutput=False)
    biasD_d = nc.declare_dram_parameter("biasD", [C, 10, W], F32, isOutput=False)
    ident_d = nc.declare_dram_parameter("ident", [32, 32], FP8, isOutput=False)
    out_d = nc.declare_dram_parameter("out", [B, C, BR, W], F32, isOutput=True)

    rg = [list(range(R))]

    with tile.TileContext(nc) as tc:
        with (
            tc.tile_pool(name="const", bufs=1) as constp,
            tc.tile_pool(name="pa", bufs=3) as pap,
            tc.tile_pool(name="persist", bufs=1) as persistp,
            tc.tile_pool(name="kv", bufs=2) as kvp,
            tc.tile_pool(name="exp", bufs=1) as ep,
            tc.tile_pool(name="small", bufs=2) as smallp,
            tc.tile_pool(name="cpadp", bufs=1) as cpadp,
            tc.tile_pool(name="phc", bufs=1) as phcp,
            tc.tile_pool(name="psum_s", bufs=1, space="PSUM") as psum_s,
            tc.tile_pool(name="psum_ctx", bufs=2, space="PSUM") as psum_ctx,
            tc.tile_pool(name="dram", bufs=1, space="DRAM") as dramp,
        ):
            pid_sp = nc.sync.partition_id()
            pid_act = nc.scalar.partition_id()

            _conv_ps_state = [0]

            def conv_psum(shape):
                _conv_ps_state[0] = (_conv_ps_state[0] + 1) % 4
                if _conv_ps_state[0] == 1:
                    return psum_s.tile(shape, F32, tag="s0", name="cps")
                if _conv_ps_state[0] == 2:
                    return psum_s.tile(shape, F32, tag="s1", name="cps")
                if _conv_ps_state[0] == 3:
                    return psum_s.tile(shape, F32, tag="s2", name="cps")
                return psum_ctx.tile(shape, F32, tag="ctx", name="cps")

            # ---------------- constants into SBUF ----------------
            # xband + phase-A weights + biasA first: they gate the phase-A
            # convs that gate the first AllGather (the critical-path head).
            xband_sb = constp.tile([CIN, B, 12, 66], F32R, tag="xband")
            nc.sync.dma_start(xband_sb[:], xband_d[:].bitcast(F32R))
            w_sb = {}
            for key, (wname, cin, _bi, is_bf) in CONVS.items():
                if is_bf:
                    t = constp.tile([cin, 9, C], BF16, tag="w" + key)
                    nc.sync.dma_start(t[:], wd[key][:])
                else:
                    t = constp.tile([cin, 9, C], F32R, tag="w" + key)
                    nc.sync.dma_start(t[:], wd[key][:].bitcast(F32R))
                w_sb[key] = t
            bias_sb = constp.tile([C, 11], F32, tag="bias")
            nc.sync.dma_start(bias_sb[:], bias_d[:])
            ident_sb = constp.tile([32, 32], FP8, tag="ident")
            nc.sync.dma_start(ident_sb[:], ident_d[:])
            biasA_sb = constp.tile([C, 3, 10, W], F32, tag="biasA")
            nc.sync.dma_start(biasA_sb[:], biasA_d[:])
            biasC_sb = constp.tile([C, 3, 12, W], F32, tag="biasC")
            nc.scalar.dma_start(biasC_sb[:], biasC_d[:])
            biasD_sb = constp.tile([C, 10, W], F32, tag="biasD")
            nc.scalar.dma_start(biasD_sb[:], biasD_d[:])

            # warm the exp table early (overlaps with phase A)
            dummy = constp.tile([1, 16], F32, tag="dummy")
            nc.vector.memset(dummy[:], 0.0)
            nc.scalar.activation(dummy[:], dummy[:], AF.Exp)

            # zero fill for the unused AllToAll shards
            zsh = constp.tile([C, R, 3 * 3 * W], BF16, tag="zsh")
            nc.gpsimd.memset(zsh[:], 0.0)
            zero_sb = constp.tile([C, 16], F32, tag="zero")
            nc.vector.memset(zero_sb[:], 0.0)

            def zcol(n):
                return zero_sb[:, 0:n].rearrange("c (a b) -> c a b", b=1).bitcast(F32R)

            def relu_bias(out_ap, psum_ap, bcol):
                # out = max(psum + bias[bcol], 0)
                nc.vector.tensor_scalar(
                    out_ap, psum_ap, bias_sb[:, bcol:bcol + 1], 0.0,
                    ALU.add, ALU.max,
                )

            def relu_img(out_ap, psum_ap, bimg_ap, tmp_tag):
                # out = max(psum + bias_image, 0) — bias image carries -1e30 on
                # out-of-image rows so the relu zeroes them.
                tmpb = smallp.tile(list(psum_ap.shape), F32, tag=tmp_tag, name="tmpb")
                nc.vector.tensor_add(tmpb[:], psum_ap, bimg_ap)
                nc.vector.tensor_scalar(out_ap, tmpb[:], 0.0, None, ALU.max)

            # ---------------- collective buffers ----------------
            contrib1 = [
                dramp.tile([KF], FP8, tag=f"c1_{s}", name=f"contrib1_{s}")
                for s in range(B)
            ]
            gath1 = [
                dramp.tile(
                    [R, KF], FP8, tag=f"g1_{s}", name=f"gath1_{s}",
                    addr_space="Shared",
                )
                for s in range(B)
            ]
            # AllToAll halo exchange: shard j of contrib2 = data addressed to
            # core j.  Shard layout [C, pass, 3 rows, W].
            contrib2 = [
                dramp.tile(
                    [R, C, 3, 3, W], BF16, tag=f"c2_{s}", name=f"contrib2_{s}"
                )
                for s in range(B)
            ]
            gath2 = [
                dramp.tile(
                    [R, C, 3, 3, W], BF16, tag=f"g2_{s}", name=f"gath2_{s}",
                )
                for s in range(B)
            ]
            for s in range(B):
                nc.sync.dma_start(
                    contrib2[s][:].rearrange("r c a b w -> c r (a b w)"),
                    zsh[:],
                )

            # ---------------- phase A: q/k/v bands ----------------
            cpad = {}   # (s, p) -> [C, 14, 66] bf16
            for s in range(B):
                for p in range(3):
                    cp = cpadp.tile([C, 14, 66], BF16, tag=f"cpad_{s}_{p}")
                    nc.vector.memset(cp[:, :, 0:1], 0.0)
                    nc.vector.memset(cp[:, :, 65:66], 0.0)
                    cpad[(s, p)] = cp

            qband = {}    # s -> [C, 3, PX] bf16 SBUF
            ksbs, vtsbs = {}, {}

            def phase_a(s):
                qb = persistp.tile([C, 3, PX], FP8, tag=f"qband_{s}")
                qband[s] = qb
                # software-pipelined: emit conv1 halves of tensor t+1
                # before conv2 of tensor t so the PE never stalls on the
                # DVE relu chain between them
                q1pads = {}

                def emit_c1_half(t, j0):
                    tn = "qkv"[t]
                    if t not in q1pads:
                        q1p = pap.tile([C, 10, 66], F32R, tag="q1pad")
                        nc.sync.dma_start(q1p[:, :, 0:1], zcol(10))
                        nc.sync.dma_start(q1p[:, :, 65:66], zcol(10))
                        q1pads[t] = q1p
                    q1p = q1pads[t]
                    ps = conv_psum([C, 5, W])
                    for tap in range(9):
                        dy, dx = divmod(tap, 3)
                        nc.tensor.matmul(
                            ps[:],
                            w_sb[tn + "1"][:, tap, :],
                            xband_sb[:, s, j0 + dy:j0 + dy + 5, dx:dx + W],
                            start=(tap == 0), stop=(tap == 8),
                        )
                    relu_img(
                        q1p[:, j0:j0 + 5, 1:65], ps[:],
                        biasA_sb[:, t, j0:j0 + 5, :], "tmpA",
                    )

                def emit_c2(t):
                    tn = "qkv"[t]
                    _, _, bi2, _ = CONVS[tn + "2"]
                    ps = conv_psum([C, BR, W])
                    for tap in range(9):
                        dy, dx = divmod(tap, 3)
                        nc.tensor.matmul(
                            ps[:],
                            w_sb[tn + "2"][:, tap, :],
                            q1pads[t][:, dy:dy + BR, dx:dx + W],
                            start=(tap == 0), stop=(tap == 8),
                        )
                    relu_bias(qb[:, t, :], ps[:], bi2)

                emit_c1_half(0, 0)
                emit_c1_half(0, 5)
                emit_c1_half(1, 0)
                emit_c2(0)
                emit_c1_half(1, 5)
                emit_c1_half(2, 0)
                emit_c2(1)
                emit_c1_half(2, 5)
                emit_c2(2)

                # second sample's contrib write goes out on the ACT queue so
                # it isn't stuck behind the s0 gather loads on SP
                ceng = nc.sync if s == 0 else nc.scalar
                ceng.dma_start(
                    contrib1[s][:].rearrange("(t c px) -> c t px", t=3, c=C, px=PX),
                    qb[:],
                )
                nc.gpsimd.collective_compute(
                    "AllGather", ALU.bypass, replica_groups=rg,
                    ins=[contrib1[s][:]], outs=[gath1[s][:]],
                )

                # gather loads + V^T rebuild, emitted immediately after the
                # collective so their (conservative) dependency snapshots
                # don't include later work
                ksb = []
                for t, eng in zip(range(3), (nc.sync, nc.sync, nc.sync)):
                    kt_ = kvp.tile([C, R, PX], FP8, tag=f"ksb{t}")
                    eng.dma_start(
                        kt_[:],
                        gath1[s][:, t * TSZ:(t + 1) * TSZ]
                        .rearrange("r (c px) -> c r px", c=C),
                    )
                    ksb.append(kt_)
                ksbs[s] = ksb

            def build_vt(s):
                ksb = ksbs[s]
                # V^T chunk forms rebuilt via XBAR DMA transposes:
                # vtsb[p, r, t, i, w] = band value [ch w, px 128*i + p] of
                # tensor t, rank r; col 32 of each 33-block is the ones column.
                vtsb = kvp.tile([128, R, 3, 4, 33], BF16, tag="vtsb")
                nc.vector.memset(vtsb[:, :, :, :, 32:33], 1.0)
                # V^T rebuilt on the PE (XBAR transposes can't do 1-byte
                # dtypes and Tile serializes them against all collectives
                # anyway); psum batches 4 chunks, one DVE copy converts to
                # bf16 for the ctx matmuls.
                for t in ((0 + 2) % 3, (1 + 2) % 3, (2 + 2) % 3):
                    for r in range(R):
                        # fp8 PE transpose requires an output element step of
                        # 2; write lane 0 of a [.., 2] psum tile
                        trp = psum_ctx.tile([128, 4, 32, 2], FP8, tag="ctx", name="trp")
                        for ip in range(4):
                            nc.tensor.transpose(
                                trp[:, ip, :, 0:1].rearrange("p w a -> p (w a)"),
                                ksb[t][:, r, 128 * ip:128 * ip + 128],
                                ident_sb[:],
                            )
                        nc.vector.tensor_copy(
                            vtsb[:, r, t, :, 0:32],
                            trp[:, :, :, 0:1].rearrange("p i w a -> p i (w a)"),
                        )
                vtsbs[s] = vtsb

            # Emission order matters: each engine's stream executes in
            # order, so the PE transposes for s0 (which wait on the first
            # gather) must come after BOTH samples' phase-A convs.
            phase_a(0)
            phase_a(1)
            build_vt(0)   # PE transposes, run while AG1(s1) is in flight

            # ---------------- phase B: attention units ----------------
            for s in range(B):
                ksb = ksbs[s]
                for p in range(3):
                    if s == 0 and p == 2:
                        # rebuild s1's V^T while the s0 units still run, so
                        # B(s1) starts with vtsb ready
                        build_vt(1)
                    vtsb = vtsbs[s]
                    kt, vt = (p + 1) % 3, (p + 2) % 3
                    qrhs = qband[s][:, p, :]

                    ctxps = psum_ctx.tile([128, PX], F32, tag="ctx")
                    ngroups = (NCH + GS - 1) // GS

                    def emit_s_group(g):
                        csz = min(GS, NCH - g * GS)
                        sps = psum_s.tile(
                            [128, GS * PX], F32, tag=f"s{g % 3}", name="sps"
                        )
                        for ci in range(csz):
                            i = g * GS + ci
                            rr, ip = divmod(i, 4)
                            nc.tensor.matmul(
                                sps[:, ci * PX:(ci + 1) * PX],
                                ksb[kt][:, rr, 128 * ip:128 * ip + 128],
                                qrhs,
                                start=True, stop=True,
                            )
                        return sps, csz

                    # software pipeline: emit S(g+1) before ctx(g) so the PE
                    # stream never blocks on exp(g) before starting S(g+1).
                    # Exp work is split between ScalarE (true exp) and DVE
                    # (Schraudolph bf16 bit-trick: bitpattern = a*x + b
                    # computed as f32 -> int16, reinterpreted as bf16; ~2%
                    # sawtooth error that washes out in the softmax average).
                    sps, csz = emit_s_group(0)
                    for g in range(ngroups):
                        es = ep.tile([128, GS * PX], BF16, tag=f"e{g % 4}")
                        if g in (12, 13, 14, 15):
                            nc.vector.tensor_scalar(
                                es[:, 0:csz * PX].bitcast(mybir.dt.int16),
                                sps[:, 0:csz * PX],
                                (128.0 / 0.6931471805599453) * SCALE, 16249.1,
                                ALU.mult, ALU.add,
                            )
                        else:
                            nc.scalar.activation(
                                es[:, 0:csz * PX], sps[:, 0:csz * PX], AF.Exp,
                                scale=SCALE,
                            )
                        cur_csz = csz
                        if g + 1 < ngroups:
                            sps, csz = emit_s_group(g + 1)
                        for ci in range(cur_csz):
                            i = g * GS + ci
                            rr, ip = divmod(i, 4)
                            nc.tensor.matmul(
                                ctxps[0:33, :],
                                vtsb[:, rr, vt, ip, :],
                                es[:, ci * PX:(ci + 1) * PX],
                                start=(i == 0), stop=(i == NCH - 1),
                            )

                    rs = smallp.tile([1, PX], F32, tag="rs")
                    nc.vector.tensor_copy(rs[:], ctxps[32:33, :])
                    recip = smallp.tile([1, PX], F32, tag="recip")
                    nc.vector.reciprocal(recip[:], rs[:])
                    bcast = smallp.tile([C, PX], F32, tag="bcast")
                    nc.gpsimd.partition_broadcast(bcast[:], recip[:])
                    cp = cpad[(s, p)]
                    nc.vector.tensor_mul(
                        cp[:, 3:11, 1:65],
                        ctxps[0:32, :].rearrange("c (a w) -> c a w", a=BR),
                        bcast[:].rearrange("c (a w) -> c a w", a=BR),
                    )
                    # edge rows of the own band, addressed to the neighbors
                    # that use them as halo: top rows -> core pid-1 (its
                    # bottom halo), bottom rows -> core pid+1 (its top halo).
                    # Edge cores skip the wraparound write so the neighbor
                    # reads SAME-padding zeros instead of wrapped data.
                    nc.sync.dma_start(
                        contrib2[s][bass.ds((pid_sp + 7) % 8, 1), :, p, :, :]
                        .rearrange("a c g w -> (a c) g w"),
                        cp[:, 3:6, 1:65],
                        cond=(pid_sp != 0),
                    )
                    nc.sync.dma_start(
                        contrib2[s][bass.ds((pid_sp + 1) % 8, 1), :, p, :, :]
                        .rearrange("a c g w -> (a c) g w"),
                        cp[:, 8:11, 1:65],
                        cond=(pid_sp != R - 1),
                    )

                nc.gpsimd.collective_compute(
                    "AllToAll", ALU.bypass, replica_groups=rg,
                    ins=[contrib2[s][:]], outs=[gath2[s][:]],
                )

            # ---------------- phase C: output convs ----------------
            for s in range(B):
                tmp = {}
                for p, pn in enumerate(("r", "g", "b")):
                    cp = cpad[(s, p)]
                    # halo rows from the AllToAll: slot (pid-1)%8 holds what
                    # the upper neighbor addressed to us (our top halo), slot
                    # (pid+1)%8 the lower neighbor's rows (our bottom halo).
                    nc.scalar.dma_start(
                        cp[:, 0:3, 1:65],
                        gath2[s][bass.ds((pid_act + 7) % 8, 1), :, p, :, :]
                        .rearrange("a c g w -> (a c) g w"),
                    )
                    nc.scalar.dma_start(
                        cp[:, 11:14, 1:65],
                        gath2[s][bass.ds((pid_act + 1) % 8, 1), :, p, :, :]
                        .rearrange("a c g w -> (a c) g w"),
                    )
                    tp = phcp.tile([C, 12, W], F32, tag=f"tmp{p}")
                    for j0 in (0, 6):
                        ps = conv_psum([C, 6, W])
                        for tap in range(9):
                            dy, dx = divmod(tap, 3)
                            nc.tensor.matmul(
                                ps[:],
                                w_sb[pn][:, tap, :],
                                cp[:, j0 + dy:j0 + dy + 6, dx:dx + W],
                                start=(tap == 0), stop=(tap == 8),
                            )
                        relu_img(
                            tp[:, j0:j0 + 6, :], ps[:],
                            biasC_sb[:, p, j0:j0 + 6, :], "tmpC",
                        )
                    tmp[p] = tp

                xctx = phcp.tile([96, 12, 66], F32R, tag="xctx")
                nc.sync.dma_start(xctx[64:96, :, 0:1], zcol(12))
                nc.sync.dma_start(xctx[64:96, :, 65:66], zcol(12))
                nc.vector.tensor_copy(
                    xctx[0:64, :, :], xband_sb[:, s, :, :].bitcast(F32)
                )
                avg = phcp.tile([C, 12, W], F32, tag="avg")
                nc.vector.tensor_add(avg[:], tmp[0][:], tmp[1][:])
                nc.vector.tensor_add(xctx[64:96, :, 1:65], avg[:], tmp[2][:])

                w2pad = phcp.tile([C, 10, 66], F32R, tag="w2pad")
                nc.sync.dma_start(w2pad[:, :, 0:1], zcol(10))
                nc.sync.dma_start(w2pad[:, :, 65:66], zcol(10))
                for j0 in (0, 5):
                    ps = conv_psum([C, 5, W])
                    for tap in range(9):
                        dy, dx = divmod(tap, 3)
                        nc.tensor.matmul(
                            ps[:],
                            w_sb["2"][:, tap, :],
                            xctx[:, j0 + dy:j0 + dy + 5, dx:dx + W],
                            start=(tap == 0), stop=(tap == 8),
                        )
                    relu_img(
                        w2pad[:, j0:j0 + 5, 1:65], ps[:],
                        biasD_sb[:, j0:j0 + 5, :], "tmpD",
                    )

                ps = conv_psum([C, BR, W])
                _, _, bi3, _ = CONVS["3"]
                for tap in range(9):
                    dy, dx = divmod(tap, 3)
                    nc.tensor.matmul(
                        ps[:],
                        w_sb["3"][:, tap, :],
                        w2pad[:, dy:dy + BR, dx:dx + W],
                        start=(tap == 0), stop=(tap == 8),
                    )
                outsb = smallp.tile([C, BR, W], F32, tag="outsb")
                relu_bias(outsb[:], ps[:], bi3)
                nc.sync.dma_start(out_d[s], outsb[:])

    nc.compile()
    return nc


def _pack_w(w, bf):
    # [Cout, Cin, 3, 3] -> lhsT pack [Cin, 9, Cout]
    w = np.asarray(w, np.float32)
    p = np.ascontiguousarray(w.transpose(1, 2, 3, 0).reshape(w.shape[1], 9, w.shape[0]))
    if bf:
        import ml_dtypes

        p = p.astype(ml_dtypes.bfloat16)
    return p


NEG = np.float32(-1e30)


def prep_in_maps(inputs):
    x = np.asarray(inputs["x"], np.float32)
    xp = np.zeros((B, CIN, H + 4, W + 2), np.float32)
    xp[:, :, 2:2 + H, 1:1 + W] = x

    shared = {}
    for key, (wname, cin, _bi, is_bf) in CONVS.items():
        w = np.asarray(inputs[wname], np.float32)
        if key == "2":
            w = w.copy()
            w[:, CIN:, :, :] /= 3.0   # fold the ctx 3-way average into w2
        shared["p_" + wname] = _pack_w(w, is_bf)
    bnames = ("bq1", "bq2", "bk1", "bk2", "bv1", "bv2", "br", "bg", "bb", "b2", "b3")
    bvals = {bn: np.asarray(inputs[bn], np.float32) for bn in bnames}
    shared["biases"] = np.ascontiguousarray(np.stack([bvals[bn] for bn in bnames], axis=1))
    import ml_dtypes

    shared["ident"] = np.eye(32, dtype=ml_dtypes.float8_e4m3fn)

    in_maps = []
    for r in range(R):
        r0 = BR * r
        xband = np.ascontiguousarray(
            xp[:, :, r0:r0 + 12, :].transpose(1, 0, 2, 3)
        )  # [CIN, B, 12, 66]

        # bias images; -1e30 rows get relu'd to the zero SAME padding expects
        biasA = np.stack(
            [np.broadcast_to(bvals[bn][:, None, None], (C, 10, W)).copy()
             for bn in ("bq1", "bk1", "bv1")], axis=1,
        )  # [C, 3, 10, W] ; conv1 out rows r0-1 .. r0+8
        biasC = np.stack(
            [np.broadcast_to(bvals[bn][:, None, None], (C, 12, W)).copy()
             for bn in ("br", "bg", "bb")], axis=1,
        )  # [C, 3, 12, W] ; wr/g/b out rows r0-2 .. r0+9
        biasD = np.broadcast_to(bvals["b2"][:, None, None], (C, 10, W)).copy()
        if r == 0:
            biasA[:, :, 0, :] = NEG
            biasC[:, :, 0:2, :] = NEG
            biasD[:, 0, :] = NEG
        if r == R - 1:
            biasA[:, :, 9, :] = NEG
            biasC[:, :, 10:12, :] = NEG
            biasD[:, 9, :] = NEG

        in_maps.append(dict(
            shared, xband=xband,
            biasA=np.ascontiguousarray(biasA),
            biasC=np.ascontiguousarray(biasC),
            biasD=np.ascontiguousarray(biasD),
        ))
    return in_maps


_CACHE = {}


def get_program():
    if "nc" not in _CACHE:
        _CACHE["nc"] = build_program()
    return _CACHE["nc"]


def kernel(**inputs):
    nc = get_program()
    in_maps = prep_in_maps(inputs)
    res = run_bass_kernel_spmd(nc, in_maps, list(range(R)))
    out = np.zeros((B, C, H, W), np.float32)
    for r in range(R):
        out[:, :, BR * r:BR * (r + 1), :] = res.results[r]["out"]
    return out
